# revision 3
# baseline (speedup 1.0000x reference)
"""Binary conv forward kernel for Trainium2 (8 NeuronCores, data-parallel over batch).

Computes y = conv2d(sign(x), scale[o] * sign(w)), stride 1, pad 1, NCHW/OIHW,
x [16, 64, 224, 224] f32, w [64*64*3*3, 1] f32 -> y [16, 64, 224, 224] f32.

Sharding: batch 16 -> 2 images per core, weights replicated (tiny).

Device algorithm (per core, n_batch=2 images):
  - Row-pair layout: SBUF "U" tiles hold sign(x) for two consecutive image rows
    (odd row 2m-1 on partitions 0..63, even row 2m on partitions 64..127, both
    batch images side by side in the free dim, with one zero pad column on each
    side for the kw shifts).
  - Each output row pair (2m, 2m+1) accumulates in PSUM [128, 2, 224] via 6
    matmuls (K=128, M=128, N=448): 3 kw-shifts against U_m with block weights
    [[W0,0],[W1,W0]] and 3 against U_{m+1} with [[W2,W1],[0,W2]], where
    Wk = sign(w)[.,.,kh=k,kw]^T in bf16 (exact since values are +-1).
  - PSUM is evicted through ScalarE with a per-partition scale[o] multiplier
    (scale = mean|w| per output channel, computed on device).
"""

import numpy as np

import concourse.bacc as bacc
import concourse.mybir as mybir
import concourse.tile as tile

F32 = mybir.dt.float32
BF16 = mybir.dt.bfloat16

N_CORES = 8
FULL_BATCH = 16
C = 64  # in channels == out channels
H = 224
W = 224
KH = KW = 3


def build_nc(n_batch=2, h=H, w=W, g=16, enable_asserts=False):
    """Build the single-core Bass module (same NEFF runs on all 8 cores)."""
    nc = bacc.Bacc(
        "TRN2",
        target_bir_lowering=False,
        debug=False,
        enable_asserts=enable_asserts,
    )
    assert h % 2 == 0
    x = nc.dram_tensor("x", [n_batch, C, h, w], F32, kind="ExternalInput")
    wraw = nc.dram_tensor("wraw", [C * C * KH * KW, 1], F32, kind="ExternalInput")
    # wt: host-permuted weight copy, layout [i, (kh kw o)] so every tap block
    # [i, o] is a contiguous [64, 64] slab (pure relayout, no arithmetic).
    wt = nc.dram_tensor("wt", [C, KH * KW * C], F32, kind="ExternalInput")
    y = nc.dram_tensor("y", [n_batch, C, h, w], F32, kind="ExternalOutput")

    NP = h // 2          # output row pairs
    NU = NP + 1          # U tiles (m = 0 .. NP)

    with tile.TileContext(nc) as tc:
        with (
            tc.tile_pool(name="wpool", bufs=1) as wpool,
            tc.tile_pool(name="icpool", bufs=2) as icpool,
            tc.tile_pool(name="upool", bufs=8) as upool,
            tc.tile_pool(name="pspool", bufs=4, space="PSUM") as pspool,
            tc.tile_pool(name="ocpool", bufs=2) as ocpool,
        ):
            # ---- weight prep (one-time, tiny) ----
            # scale[o] = mean(|w[o, :, :, :]|), built with O on partitions,
            # duplicated on both partition halves for the [128]-row eviction.
            w2 = wpool.tile([128, 576], F32)
            wr = wraw.ap().rearrange("(o f) one -> o (f one)", o=C)
            nc.sync.dma_start(w2[0:64], wr)
            nc.sync.dma_start(w2[64:128], wr)
            absw = wpool.tile([128, 576], F32)
            sc_sum = wpool.tile([128, 1], F32)
            nc.scalar.activation(
                out=absw[:], in_=w2[:], func=mybir.ActivationFunctionType.Abs,
                accum_out=sc_sum[:],
            )
            sc128 = wpool.tile([128, 1], F32)
            nc.scalar.mul(sc128[:], sc_sum[:], 1.0 / 576.0)

            # sign(w)^T in bf16, duplicated on both partition halves.
            wtf = wpool.tile([128, 576], F32)
            nc.sync.dma_start(wtf[0:64], wt.ap())
            nc.sync.dma_start(wtf[64:128], wt.ap())
            swt = wpool.tile([128, 576], BF16)
            nc.scalar.sign(swt[:], wtf[:])

            def tap(kh, kw):
                o0 = (kh * 3 + kw) * 64
                return slice(o0, o0 + 64)

            # Block weight matrices for the row-pair matmuls.
            lo = []
            hi = []
            for kw in range(3):
                lot = wpool.tile([128, 128], BF16, name=f"lo{kw}")
                nc.vector.memset(lot[:], 0.0)
                # [[W0, 0], [W1, W0]]  (rows = K halves, cols = M halves)
                nc.vector.tensor_copy(out=lot[0:64, 0:64], in_=swt[0:64, tap(0, kw)])
                nc.vector.tensor_copy(out=lot[64:128, 0:64], in_=swt[64:128, tap(1, kw)])
                nc.vector.tensor_copy(out=lot[64:128, 64:128], in_=swt[64:128, tap(0, kw)])
                lo.append(lot)
                hit = wpool.tile([128, 128], BF16, name=f"hi{kw}")
                nc.vector.memset(hit[:], 0.0)
                # [[W2, W1], [0, W2]]
                nc.vector.tensor_copy(out=hit[0:64, 0:64], in_=swt[0:64, tap(2, kw)])
                nc.vector.tensor_copy(out=hit[0:64, 64:128], in_=swt[0:64, tap(1, kw)])
                nc.vector.tensor_copy(out=hit[64:128, 64:128], in_=swt[64:128, tap(2, kw)])
                hi.append(hit)

            # ---- main loop over row pairs ----
            # x viewed as [c, n, h/2, parity, w]: parity 0 = even rows, 1 = odd.
            xr = x.ap().rearrange("n c (hh two) w -> c n hh two w", two=2)
            yr = y.ap().rearrange("n c (hh two) w -> c n hh two w", two=2)

            ic = None           # current input chunk tile
            ic_u0 = 0           # first U index covered by ic
            oc = None           # current output chunk tile
            oc_p0 = 0           # first pair covered by oc
            u_prev = None       # U tile for m-1

            for u in range(NU):
                if u % g == 0:
                    # Load input chunk covering U indices [u, u+gc).
                    gc = min(g, NU - u)
                    ic = icpool.tile([128, n_batch, g, w], F32, tag="ic", name="ic")
                    ic_u0 = u
                    # DMA engines only support 3 total dims after merging, so
                    # issue one DMA per batch image per half.
                    for b in range(n_batch):
                        # top half: odd rows 2u-1 .. (odd idx u-1 .. u+gc-2)
                        if u == 0:
                            if gc > 1:
                                nc.sync.dma_start(
                                    ic[0:64, b, 1:gc, :], xr[:, b, 0 : gc - 1, 1, :]
                                )
                        else:
                            nc.sync.dma_start(
                                ic[0:64, b, 0:gc, :],
                                xr[:, b, u - 1 : u + gc - 1, 1, :],
                            )
                        # bottom half: even rows 2u .. (even idx u .. u+gc-1,
                        # capped: even index NP does not exist)
                        gb = min(gc, NP - u)
                        if gb > 0:
                            nc.sync.dma_start(
                                ic[64:128, b, 0:gb, :], xr[:, b, u : u + gb, 0, :]
                            )

                # Build sign tile U_u [128, n_batch, w + 2] (pad col each side).
                uu = upool.tile([128, n_batch, w + 2], BF16, tag="U", name="uu")
                nc.vector.memset(uu[:, :, 0:1], 0.0)
                nc.vector.memset(uu[:, :, w + 1 : w + 2], 0.0)
                j = u - ic_u0
                if u == 0:
                    nc.vector.memset(uu[0:64, :, 1 : w + 1], 0.0)
                    nc.scalar.sign(uu[64:128, :, 1 : w + 1], ic[64:128, :, j, :])
                elif u == NP:
                    nc.scalar.sign(uu[0:64, :, 1 : w + 1], ic[0:64, :, j, :])
                    nc.vector.memset(uu[64:128, :, 1 : w + 1], 0.0)
                else:
                    nc.scalar.sign(uu[:, :, 1 : w + 1], ic[:, :, j, :])

                if u >= 1:
                    m = u - 1  # output pair index
                    if m % g == 0:
                        oc = ocpool.tile(
                            [128, n_batch, g, w], F32, tag="oc", name="oc"
                        )
                        oc_p0 = m
                    ps = pspool.tile([128, n_batch, w], F32, tag="ps", name="ps")
                    for kw in range(3):
                        nc.tensor.matmul(
                            ps[:], lo[kw][:], u_prev[:, :, kw : kw + w],
                            start=(kw == 0), stop=False,
                        )
                    for kw in range(3):
                        nc.tensor.matmul(
                            ps[:], hi[kw][:], uu[:, :, kw : kw + w],
                            start=False, stop=(kw == 2),
                        )
                    # Evict with per-channel scale.
                    nc.scalar.mul(oc[:, :, m - oc_p0, :], ps[:], sc128[:])

                    if m == oc_p0 + g - 1 or m == NP - 1:
                        cnt = m - oc_p0 + 1
                        for b in range(n_batch):
                            nc.sync.dma_start(
                                yr[:, b, oc_p0 : m + 1, 0, :], oc[0:64, b, 0:cnt, :]
                            )
                            nc.sync.dma_start(
                                yr[:, b, oc_p0 : m + 1, 1, :], oc[64:128, b, 0:cnt, :]
                            )

                u_prev = uu

    nc.compile()
    return nc


_NC_CACHE = {}


def _get_nc(key=(2, H, W, 16)):
    if key not in _NC_CACHE:
        _NC_CACHE[key] = build_nc(*key)
    return _NC_CACHE[key]


def _make_wt(weights):
    # [o*i*kh*kw, 1] -> [i, (kh kw o)] contiguous (pure relayout on host)
    w4 = np.ascontiguousarray(
        weights.reshape(C, C, KH, KW).transpose(1, 2, 3, 0)
    ).reshape(C, KH * KW * C)
    return w4.astype(np.float32, copy=False)


def kernel(x, weights):
    from concourse import bass_utils

    x = np.asarray(x, dtype=np.float32)
    weights = np.asarray(weights, dtype=np.float32)
    nc = _get_nc()
    wt = _make_wt(weights)
    nb = FULL_BATCH // N_CORES
    in_maps = [
        {"x": x[c * nb : (c + 1) * nb], "wraw": weights, "wt": wt}
        for c in range(N_CORES)
    ]
    res = bass_utils.run_bass_kernel_spmd(nc, in_maps, core_ids=list(range(N_CORES)))
    return np.concatenate([r["y"] for r in res.results], axis=0)


# revision 5
# speedup vs baseline: 1.4174x; 1.4174x over previous
"""Binary conv forward kernel for Trainium2 (8 NeuronCores, data-parallel over batch).

Computes y = conv2d(sign(x), scale[o] * sign(w)), stride 1, pad 1, NCHW/OIHW,
x [16, 64, 224, 224] f32, w [64*64*3*3, 1] f32 -> y [16, 64, 224, 224] f32.

Sharding: batch 16 -> 2 images per core, weights replicated (tiny).

Host side packs x into a tile-major layout [pair, 128, batch, w] (pure
relayout) so that every device DMA is a dense 128-partition 3-dim copy running
at full 16-SDMA-engine rate; the output is produced in a matching packed
layout and unpacked after the gather.

Device algorithm (per core, n_batch=2 images):
  - V_j tiles hold sign(x) for rows (2j, 2j+1): even row on partitions 0..63,
    odd row on partitions 64..127, both batch images in the free dim, one zero
    pad column each side for the kw shifts.  sign() runs on ScalarE (exact in
    bf16 since values are -1/0/+1).
  - Interior output pair (2m+1, 2m+2) accumulates in PSUM [128, 2, 224] via 6
    matmuls (K=128, M=128, N=448): 3 kw-shifts against V_m with block weights
    [[W0,0],[W1,W0]] and 3 against V_{m+1} with [[W2,W1],[0,W2]], where
    Wk = sign(w)[:,:,kh=k,kw]^T (bf16).  Boundary rows 0 and 223 form one
    extra unit using block weights [[W1,0],[W2,0]] (rhs V_0) and
    [[0,W0],[0,W1]] (rhs V_111).
  - PSUM is evicted through VectorE with a per-partition scale[o] multiplier
    (scale = mean|w| per output channel, computed on device).
  - Input DMAs ride the HWDGE ring (nc.sync); output DMAs ride SWDGE
    (nc.gpsimd) so loads and stores overlap on HBM.
"""

import numpy as np

import concourse.bacc as bacc
import concourse.mybir as mybir
import concourse.tile as tile

F32 = mybir.dt.float32
BF16 = mybir.dt.bfloat16

N_CORES = 8
FULL_BATCH = 16
C = 64  # in channels == out channels
H = 224
W = 224
KH = KW = 3


def build_nc(n_batch=2, h=H, w=W, g=16, enable_asserts=False):
    """Build the single-core Bass module (same NEFF runs on all 8 cores)."""
    nc = bacc.Bacc(
        "TRN2",
        target_bir_lowering=False,
        debug=False,
        enable_asserts=enable_asserts,
    )
    assert h % 2 == 0
    NV = h // 2          # V tiles (input row pairs), also output units
    NI = NV - 1          # interior output units

    # Packed tensors: [pair, 128, batch, w].
    xp = nc.dram_tensor("xp", [NV, 128, n_batch, w], F32, kind="ExternalInput")
    wraw = nc.dram_tensor("wraw", [C * C * KH * KW, 1], F32, kind="ExternalInput")
    # wt: host-permuted weight copy, layout [i, (kh kw o)] so every tap block
    # [i, o] is a contiguous [64, 64] slab (pure relayout, no arithmetic).
    wt = nc.dram_tensor("wt", [C, KH * KW * C], F32, kind="ExternalInput")
    yp = nc.dram_tensor("yp", [NV, 128, n_batch, w], F32, kind="ExternalOutput")

    xr = xp.ap().rearrange("j p b w -> p j (b w)")   # [128, NV, n_batch*w]
    yr = yp.ap().rearrange("j p b w -> p j (b w)")

    with tile.TileContext(nc) as tc:
        with (
            tc.tile_pool(name="wpool", bufs=1) as wpool,
            tc.tile_pool(name="icpool", bufs=3) as icpool,
            tc.tile_pool(name="upool", bufs=8) as upool,
            tc.tile_pool(name="pspool", bufs=6, space="PSUM") as pspool,
            tc.tile_pool(name="ocpool", bufs=2) as ocpool,
        ):
            # ---- weight prep (one-time, tiny) ----
            # scale[o] = mean(|w[o, :, :, :]|), O on partitions, duplicated on
            # both partition halves for the [128]-row eviction.
            w2 = wpool.tile([128, 576], F32)
            wr = wraw.ap().rearrange("(o f) one -> o (f one)", o=C)
            nc.sync.dma_start(w2[0:64], wr)
            nc.sync.dma_start(w2[64:128], wr)
            absw = wpool.tile([128, 576], F32)
            sc_sum = wpool.tile([128, 1], F32)
            nc.scalar.activation(
                out=absw[:], in_=w2[:], func=mybir.ActivationFunctionType.Abs,
                accum_out=sc_sum[:],
            )
            sc128 = wpool.tile([128, 1], F32)
            nc.scalar.mul(sc128[:], sc_sum[:], 1.0 / 576.0)

            # sign(w)^T in bf16, duplicated on both partition halves.
            wtf = wpool.tile([128, 576], F32)
            nc.sync.dma_start(wtf[0:64], wt.ap())
            nc.sync.dma_start(wtf[64:128], wt.ap())
            swt = wpool.tile([128, 576], BF16)
            nc.scalar.sign(swt[:], wtf[:])

            def tap(kh, kw):
                o0 = (kh * 3 + kw) * 64
                return slice(o0, o0 + 64)

            # Block weight matrices (rows = K halves, cols = M halves).
            lo, hi, b0, b1 = [], [], [], []
            for kw in range(3):
                lot = wpool.tile([128, 128], BF16, name=f"lo{kw}")
                nc.vector.memset(lot[:], 0.0)
                # [[W0, 0], [W1, W0]]
                nc.vector.tensor_copy(out=lot[0:64, 0:64], in_=swt[0:64, tap(0, kw)])
                nc.vector.tensor_copy(out=lot[64:128, 0:64], in_=swt[64:128, tap(1, kw)])
                nc.vector.tensor_copy(out=lot[64:128, 64:128], in_=swt[64:128, tap(0, kw)])
                lo.append(lot)
                hit = wpool.tile([128, 128], BF16, name=f"hi{kw}")
                nc.vector.memset(hit[:], 0.0)
                # [[W2, W1], [0, W2]]
                nc.vector.tensor_copy(out=hit[0:64, 0:64], in_=swt[0:64, tap(2, kw)])
                nc.vector.tensor_copy(out=hit[0:64, 64:128], in_=swt[0:64, tap(1, kw)])
                nc.vector.tensor_copy(out=hit[64:128, 64:128], in_=swt[64:128, tap(2, kw)])
                hi.append(hit)
                b0t = wpool.tile([128, 128], BF16, name=f"b0{kw}")
                nc.vector.memset(b0t[:], 0.0)
                # [[W1, 0], [W2, 0]]  (row 0 of the image)
                nc.vector.tensor_copy(out=b0t[0:64, 0:64], in_=swt[0:64, tap(1, kw)])
                nc.vector.tensor_copy(out=b0t[64:128, 0:64], in_=swt[64:128, tap(2, kw)])
                b0.append(b0t)
                b1t = wpool.tile([128, 128], BF16, name=f"b1{kw}")
                nc.vector.memset(b1t[:], 0.0)
                # [[0, W0], [0, W1]]  (row h-1 of the image)
                nc.vector.tensor_copy(out=b1t[0:64, 64:128], in_=swt[0:64, tap(0, kw)])
                nc.vector.tensor_copy(out=b1t[64:128, 64:128], in_=swt[64:128, tap(1, kw)])
                b1.append(b1t)

            # Persistent copy of V_0 for the boundary unit at the end.
            v0c = wpool.tile([128, n_batch, w + 2], BF16)

            def make_unit(ps, oc, jj, rhs_a, lhst_a, rhs_b, lhst_b):
                """One output unit: 6 accumulating matmuls + scaled evict."""
                for kw in range(3):
                    nc.tensor.matmul(
                        ps[:], lhst_a[kw][:], rhs_a[:, :, kw : kw + w],
                        start=(kw == 0), stop=False,
                    )
                for kw in range(3):
                    nc.tensor.matmul(
                        ps[:], lhst_b[kw][:], rhs_b[:, :, kw : kw + w],
                        start=False, stop=(kw == 2),
                    )
                nc.vector.tensor_scalar_mul(
                    oc[:, jj, :].rearrange("p (b w) -> p b w", b=n_batch),
                    ps[:],
                    sc128[:],
                )

            oc = None
            oc_m0 = 0
            v_prev = None

            for j in range(NV):
                if j % g == 0:
                    gc = min(g, NV - j)
                    ic = icpool.tile([128, g, n_batch * w], F32, tag="ic", name="ic")
                    nc.sync.dma_start(ic[:, 0:gc, :], xr[:, j : j + gc, :])

                vv = upool.tile([128, n_batch, w + 2], BF16, tag="V", name="vv")
                nc.vector.memset(vv[:, :, 0:1], 0.0)
                nc.vector.memset(vv[:, :, w + 1 : w + 2], 0.0)
                nc.scalar.sign(
                    vv[:, :, 1 : w + 1],
                    ic[:, j - (j // g) * g, :].rearrange("p (b w) -> p b w", b=n_batch),
                )
                if j == 0:
                    nc.vector.tensor_copy(out=v0c[:], in_=vv[:])

                if j >= 1:
                    m = j - 1  # interior unit -> output rows (2m+1, 2m+2)
                    if m % g == 0:
                        oc = ocpool.tile(
                            [128, g, n_batch * w], F32, tag="oc", name="oc"
                        )
                        oc_m0 = m
                    ps = pspool.tile([128, n_batch, w], F32, tag="ps", name="ps")
                    make_unit(ps, oc, m - oc_m0, v_prev, lo, vv, hi)

                    if m == oc_m0 + g - 1:
                        nc.gpsimd.dma_start(
                            yr[:, oc_m0 : m + 1, :], oc[:, 0:g, :]
                        )
                v_prev = vv

            # Boundary unit (unit NV-1): rows 0 and h-1.
            m = NV - 1
            if m % g == 0:
                oc = ocpool.tile([128, g, n_batch * w], F32, tag="oc", name="oc")
                oc_m0 = m
            ps = pspool.tile([128, n_batch, w], F32, tag="ps", name="ps")
            make_unit(ps, oc, m - oc_m0, v0c, b0, v_prev, b1)
            cnt = m - oc_m0 + 1
            nc.gpsimd.dma_start(yr[:, oc_m0 : m + 1, :], oc[:, 0:cnt, :])

    nc.compile()
    return nc


_NC_CACHE = {}


def _get_nc(key=(2, H, W, 16)):
    if key not in _NC_CACHE:
        _NC_CACHE[key] = build_nc(*key)
    return _NC_CACHE[key]


def _make_wt(weights):
    # [o*i*kh*kw, 1] -> [i, (kh kw o)] contiguous (pure relayout on host)
    w4 = np.ascontiguousarray(
        weights.reshape(C, C, KH, KW).transpose(1, 2, 3, 0)
    ).reshape(C, KH * KW * C)
    return w4.astype(np.float32, copy=False)


def pack_x(x_shard, h=H, w=W):
    """[nb, C, h, w] -> [h/2, 128, nb, w]; p = parity*64 + channel."""
    nb = x_shard.shape[0]
    xs = x_shard.reshape(nb, C, h // 2, 2, w)
    return np.ascontiguousarray(xs.transpose(2, 3, 1, 0, 4)).reshape(
        h // 2, 128, nb, w
    )


def unpack_y(ypk, h=H, w=W):
    """[h/2, 128, nb, w] -> [nb, C, h, w] per the unit layout."""
    NV = h // 2
    nb = ypk.shape[2]
    y = np.empty((nb, C, h, w), np.float32)
    # interior units m=0..NV-2 -> rows 2m+1 (p<64) and 2m+2 (p>=64)
    interior = ypk[: NV - 1].reshape(NV - 1, 2, C, nb, w)
    y[:, :, 1 : h - 1, :] = interior.transpose(3, 2, 0, 1, 4).reshape(
        nb, C, h - 2, w
    )
    y[:, :, 0, :] = ypk[NV - 1, 0:C].transpose(1, 0, 2)
    y[:, :, h - 1, :] = ypk[NV - 1, C:128].transpose(1, 0, 2)
    return y


def make_in_maps(x, weights):
    x = np.asarray(x, dtype=np.float32)
    weights = np.asarray(weights, dtype=np.float32)
    wt = _make_wt(weights)
    nb = FULL_BATCH // N_CORES
    return [
        {
            "xp": pack_x(x[c * nb : (c + 1) * nb]),
            "wraw": weights,
            "wt": wt,
        }
        for c in range(N_CORES)
    ]


def gather_out(results):
    return np.concatenate([unpack_y(r["yp"]) for r in results], axis=0)


def kernel(x, weights):
    from concourse import bass_utils

    nc = _get_nc()
    in_maps = make_in_maps(x, weights)
    res = bass_utils.run_bass_kernel_spmd(nc, in_maps, core_ids=list(range(N_CORES)))
    return gather_out(res.results)


# revision 16
# speedup vs baseline: 1.6792x; 1.1847x over previous
"""Binary conv forward kernel for Trainium2 (8 NeuronCores, data-parallel over batch).

Computes y = conv2d(sign(x), scale[o] * sign(w)), stride 1, pad 1, NCHW/OIHW,
x [16, 64, 224, 224] f32, w [64*64*3*3, 1] f32 -> y [16, 64, 224, 224] f32.

Sharding: batch 16 -> 2 images per core, weights replicated (tiny).

Host side packs x into a tile-major layout [pair, 128, batch, w] (pure
relayout) so that every device DMA is a dense 128-partition 3-dim copy running
at full 16-SDMA-engine rate; the output is produced in a matching packed
layout and unpacked after the gather.

Device algorithm (per core, n_batch=2 images):
  - A resident fp8 "sign plane" holds sign(x) for the whole shard: slot j =
    rows (2j, 2j+1) (even row on partitions 0..63, odd on 64..127), both batch
    images in the free dim, one zero pad column each side for the kw shifts.
    sign() runs on ScalarE; -1/0/+1 are exact in fp8e4.
  - Interior output pair (2m+1, 2m+2) accumulates in PSUM [128, 2, 224] via 3
    DoubleRow matmuls (virtual K=256 over slots m, m+1; M=128; N=448), one per
    kw shift.  The stationary operand stacks the two block matrices
    [[W0,0],[W1,W0]] (slot m) and [[W2,W1],[0,W2]] (slot m+1), where
    Wk = sign(w)[:,:,kh=k,kw]^T.  Boundary rows 0 and 223 form one extra unit
    over slots 111 (V_111) and 112 (a copy of V_0) with blocks
    [[0,W0],[0,W1]] and [[W1,0],[W2,0]].
  - Two units share one 2-bank PSUM tile; VectorE evicts both at once with a
    per-partition scale[o] multiplier (scale = mean|w| per output channel,
    computed on device).
  - Input DMAs ride the HWDGE ring (nc.sync); weight + output DMAs ride SWDGE
    (nc.gpsimd) so loads and stores overlap on HBM.
"""

import numpy as np

import concourse.bacc as bacc
import concourse.mybir as mybir
import concourse.tile as tile

F32 = mybir.dt.float32
FP8 = mybir.dt.float8e4

N_CORES = 8
FULL_BATCH = 16
C = 64  # in channels == out channels
H = 224
W = 224
KH = KW = 3
# Sign-plane slot layout (fp8, per partition): [pad, b0 w=224, 0, b1 w=224,
# pad, pad] -> both batch images form one contiguous 450-wide matmul N strip;
# the shared zero column between them keeps the kw shifts exact.  452 cols
# used, padded to 464 (multiple of 16 for DoubleRow AP steps).
SW = 464   # slot stride
SN = 450   # matmul N (448 real output columns + 2 junk)
B0 = 1     # b0 image at cols 1..224
B1 = 226   # b1 image at cols 226..449


def build_nc(n_batch=2, h=H, w=W, g=16, enable_asserts=False):
    """Build the single-core Bass module (same NEFF runs on all 8 cores)."""
    nc = bacc.Bacc(
        "TRN2",
        target_bir_lowering=False,
        debug=False,
        enable_asserts=enable_asserts,
    )
    assert h % 2 == 0
    NV = h // 2          # input row-pair slots, also output units
    assert NV % 2 == 0, "units are evicted in pairs"

    # Packed tensors: [pair, 128, batch, w].
    xp = nc.dram_tensor("xp", [NV, 128, n_batch, w], F32, kind="ExternalInput")
    wraw = nc.dram_tensor("wraw", [C * C * KH * KW, 1], F32, kind="ExternalInput")
    # wt: host-permuted weight copy, layout [i, (kh kw o)] so every tap block
    # [i, o] is a contiguous [64, 64] slab (pure relayout, no arithmetic).
    wt = nc.dram_tensor("wt", [C, KH * KW * C], F32, kind="ExternalInput")
    yp = nc.dram_tensor("yp", [NV, 128, n_batch, w], F32, kind="ExternalOutput")

    xr = xp.ap().rearrange("j p b w -> p j (b w)")   # [128, NV, n_batch*w]
    yr = yp.ap().rearrange("j p b w -> p j (b w)")

    with tile.TileContext(nc) as tc:
        with (
            tc.tile_pool(name="wpool", bufs=1) as wpool,
            tc.tile_pool(name="icpool", bufs=2) as icpool,
            tc.tile_pool(name="pspool", bufs=3, space="PSUM") as pspool,
            tc.tile_pool(name="ocpool", bufs=2) as ocpool,
        ):
            # Prefetch the first input chunk before anything else so the HWDGE
            # ring starts moving data immediately.
            gc0 = min(g, NV)
            ic = icpool.tile([128, g, n_batch * w], F32, tag="ic", name="ic")
            nc.sync.dma_start(ic[:, 0:gc0, :], xr[:, 0:gc0, :])

            # ---- weight prep (one-time, tiny; DMAs via SWDGE) ----
            # scale[o] = mean(|w[o, :, :, :]|), O on partitions, duplicated on
            # both partition halves for the [128]-row eviction.
            w2 = wpool.tile([128, 576], F32)
            wr = wraw.ap().rearrange("(o f) one -> o (f one)", o=C)
            nc.gpsimd.dma_start(w2[0:64], wr)
            nc.gpsimd.dma_start(w2[64:128], wr)
            absw = wpool.tile([128, 576], F32)
            sc_sum = wpool.tile([128, 1], F32)
            nc.scalar.activation(
                out=absw[:], in_=w2[:], func=mybir.ActivationFunctionType.Abs,
                accum_out=sc_sum[:],
            )
            sc128 = wpool.tile([128, 1], F32)
            nc.scalar.mul(sc128[:], sc_sum[:], 1.0 / 576.0)

            # sign(w)^T in fp8, duplicated on both partition halves.
            wtf = wpool.tile([128, 576], F32)
            nc.gpsimd.dma_start(wtf[0:64], wt.ap())
            nc.gpsimd.dma_start(wtf[64:128], wt.ap())
            swt = wpool.tile([128, 576], FP8)
            nc.scalar.sign(swt[:], wtf[:])

            def tap(kh, kw):
                o0 = (kh * 3 + kw) * 64
                return slice(o0, o0 + 64)

            # DoubleRow stationary operands [128, 2, 128]: index 1 of the
            # middle dim is the second virtual-K block.
            def cp(dst, src):
                nc.vector.tensor_copy(out=dst, in_=src)

            wdr, wb = [], []
            for kw in range(3):
                t = wpool.tile([128, 2, 128], FP8, name=f"wdr{kw}")
                nc.vector.memset(t[:], 0.0)
                # block 0 (slot m):   [[W0, 0], [W1, W0]]
                cp(t[0:64, 0, 0:64], swt[0:64, tap(0, kw)])
                cp(t[64:128, 0, 0:64], swt[64:128, tap(1, kw)])
                cp(t[64:128, 0, 64:128], swt[64:128, tap(0, kw)])
                # block 1 (slot m+1): [[W2, W1], [0, W2]]
                cp(t[0:64, 1, 0:64], swt[0:64, tap(2, kw)])
                cp(t[0:64, 1, 64:128], swt[0:64, tap(1, kw)])
                cp(t[64:128, 1, 64:128], swt[64:128, tap(2, kw)])
                wdr.append(t)
                tb = wpool.tile([128, 2, 128], FP8, name=f"wb{kw}")
                nc.vector.memset(tb[:], 0.0)
                # block 0 (slot NV-1 = V_{NV-1}): [[0, W0], [0, W1]] (row h-1)
                cp(tb[0:64, 0, 64:128], swt[0:64, tap(0, kw)])
                cp(tb[64:128, 0, 64:128], swt[64:128, tap(1, kw)])
                # block 1 (slot NV = copy of V_0): [[W1, 0], [W2, 0]] (row 0)
                cp(tb[0:64, 1, 0:64], swt[0:64, tap(1, kw)])
                cp(tb[64:128, 1, 0:64], swt[64:128, tap(2, kw)])
                wb.append(tb)

            # Resident sign plane [128, NV+1, SW] fp8; slot NV = V_0 copy.
            assert n_batch == 2
            plane = wpool.tile([128, NV + 1, SW], FP8)
            # Zero the pad columns once (plane slots are written once):
            # col 0 (left pad), col 225 (separator / b0 right pad), cols
            # 450-451 (right pads, also read by the junk output column).
            nc.vector.memset(plane[:, :, 0:1], 0.0)
            nc.vector.memset(plane[:, :, 225:226], 0.0)
            nc.vector.memset(plane[:, :, 450:452], 0.0)

            def rhs(j, kw):
                return plane[:, j : j + 2, kw : kw + SN]

            def evict(ps, oc, jj):
                # psum cols 0..223 = b0, 225..448 = b1 (stride-225 blocks)
                nc.vector.tensor_scalar_mul(
                    oc[:, jj : jj + 2, :].rearrange(
                        "p j (b w) -> p j b w", b=n_batch
                    ),
                    ps[:, :, 0:450].rearrange("p u (b w) -> p u b w", w=225)[
                        :, :, :, 0:w
                    ],
                    sc128[:],
                )

            oc = None
            oc_m0 = 0
            ps = None

            for j in range(NV):
                if j % g == 0 and j > 0:
                    gc = min(g, NV - j)
                    ic = icpool.tile([128, g, n_batch * w], F32, tag="ic", name="ic")
                    nc.sync.dma_start(ic[:, 0:gc, :], xr[:, j : j + gc, :])

                nc.scalar.sign(
                    plane[:, j, 1:451].rearrange("p (b w) -> p b w", w=225)[
                        :, :, 0:w
                    ],
                    ic[:, j % g, :].rearrange("p (b w) -> p b w", b=n_batch),
                )
                if j == 0:
                    cp(plane[:, NV, 0:452], plane[:, 0, 0:452])

                if j >= 1:
                    m = j - 1  # interior unit -> output rows (2m+1, 2m+2)
                    if m % g == 0:
                        oc = ocpool.tile(
                            [128, g, n_batch * w], F32, tag="oc", name="oc"
                        )
                        oc_m0 = m
                    if m % 2 == 0:
                        # per-unit stride padded to one full PSUM bank (2 KB)
                        ps = pspool.tile([128, 2, 512], F32, tag="ps", name="ps")
                    for kw in range(3):
                        nc.tensor.matmul(
                            ps[:, m % 2, 0:SN], wdr[kw][:], rhs(m, kw),
                            start=(kw == 0), stop=(kw == 2),
                            perf_mode=mybir.MatmulPerfMode.DoubleRow,
                        )
                    if m % 2 == 1:
                        evict(ps, oc, m - 1 - oc_m0)
                    if m == oc_m0 + g - 1:
                        nc.gpsimd.dma_start(yr[:, oc_m0 : m + 1, :], oc[:, 0:g, :])

            # Boundary unit (unit NV-1): rows 0 and h-1 via slots NV-1 and NV.
            m = NV - 1
            for kw in range(3):
                nc.tensor.matmul(
                    ps[:, m % 2, 0:SN], wb[kw][:], rhs(NV - 1, kw),
                    start=(kw == 0), stop=(kw == 2),
                    perf_mode=mybir.MatmulPerfMode.DoubleRow,
                )
            evict(ps, oc, m - 1 - oc_m0)
            nc.gpsimd.dma_start(yr[:, oc_m0 : m + 1, :], oc[:, 0 : m - oc_m0 + 1, :])

    nc.compile()
    return nc


_NC_CACHE = {}


def _get_nc(key=(2, H, W, 16)):
    if key not in _NC_CACHE:
        _NC_CACHE[key] = build_nc(*key)
    return _NC_CACHE[key]


def _make_wt(weights):
    # [o*i*kh*kw, 1] -> [i, (kh kw o)] contiguous (pure relayout on host)
    w4 = np.ascontiguousarray(
        weights.reshape(C, C, KH, KW).transpose(1, 2, 3, 0)
    ).reshape(C, KH * KW * C)
    return w4.astype(np.float32, copy=False)


def pack_x(x_shard, h=H, w=W):
    """[nb, C, h, w] -> [h/2, 128, nb, w]; p = parity*64 + channel."""
    nb = x_shard.shape[0]
    xs = x_shard.reshape(nb, C, h // 2, 2, w)
    return np.ascontiguousarray(xs.transpose(2, 3, 1, 0, 4)).reshape(
        h // 2, 128, nb, w
    )


def unpack_y(ypk, h=H, w=W):
    """[h/2, 128, nb, w] -> [nb, C, h, w] per the unit layout."""
    NV = h // 2
    nb = ypk.shape[2]
    y = np.empty((nb, C, h, w), np.float32)
    # interior units m=0..NV-2 -> rows 2m+1 (p<64) and 2m+2 (p>=64)
    interior = ypk[: NV - 1].reshape(NV - 1, 2, C, nb, w)
    y[:, :, 1 : h - 1, :] = interior.transpose(3, 2, 0, 1, 4).reshape(
        nb, C, h - 2, w
    )
    # boundary unit: p<64 -> row 0, p>=64 -> row h-1
    y[:, :, 0, :] = ypk[NV - 1, 0:C].transpose(1, 0, 2)
    y[:, :, h - 1, :] = ypk[NV - 1, C:128].transpose(1, 0, 2)
    return y


def make_in_maps(x, weights):
    x = np.asarray(x, dtype=np.float32)
    weights = np.asarray(weights, dtype=np.float32)
    wt = _make_wt(weights)
    nb = FULL_BATCH // N_CORES
    return [
        {
            "xp": pack_x(x[c * nb : (c + 1) * nb]),
            "wraw": weights,
            "wt": wt,
        }
        for c in range(N_CORES)
    ]


def gather_out(results):
    return np.concatenate([unpack_y(r["yp"]) for r in results], axis=0)


def kernel(x, weights):
    from concourse import bass_utils

    nc = _get_nc()
    in_maps = make_in_maps(x, weights)
    res = bass_utils.run_bass_kernel_spmd(nc, in_maps, core_ids=list(range(N_CORES)))
    return gather_out(res.results)


# revision 21
# speedup vs baseline: 1.8326x; 1.0913x over previous
"""Binary conv forward kernel for Trainium2 (8 NeuronCores, data-parallel over batch).

Computes y = conv2d(sign(x), scale[o] * sign(w)), stride 1, pad 1, NCHW/OIHW,
x [16, 64, 224, 224] f32, w [64*64*3*3, 1] f32 -> y [16, 64, 224, 224] f32.

Sharding: batch 16 -> 2 images per core, weights replicated (tiny).

Host side packs x into a tile-major layout [pair, 128, batch, w] (pure
relayout) so that every device DMA is a dense 128-partition 3-dim copy running
at full 16-SDMA-engine rate; the output is produced in a matching packed
layout and unpacked after the gather.

Device algorithm (per core, n_batch=2 images):
  - A resident fp8 "sign plane" holds sign(x) for the whole shard: slot j =
    rows (2j, 2j+1) (even row on partitions 0..63, odd on 64..127), both batch
    images in the free dim, one zero pad column each side for the kw shifts.
    sign() runs on ScalarE; -1/0/+1 are exact in fp8e4.
  - Interior output pair (2m+1, 2m+2) accumulates in PSUM [128, 2, 224] via 3
    DoubleRow matmuls (virtual K=256 over slots m, m+1; M=128; N=448), one per
    kw shift.  The stationary operand stacks the two block matrices
    [[W0,0],[W1,W0]] (slot m) and [[W2,W1],[0,W2]] (slot m+1), where
    Wk = sign(w)[:,:,kh=k,kw]^T.  Boundary rows 0 and 223 form one extra unit
    over slots 111 (V_111) and 112 (a copy of V_0) with blocks
    [[0,W0],[0,W1]] and [[W1,0],[W2,0]].
  - Two units share one 2-bank PSUM tile; VectorE evicts both at once with a
    per-partition scale[o] multiplier (scale = mean|w| per output channel,
    computed on device).
  - Input DMAs ride the HWDGE ring (nc.sync); weight + output DMAs ride SWDGE
    (nc.gpsimd) so loads and stores overlap on HBM.
"""

import numpy as np

import concourse.bacc as bacc
import concourse.mybir as mybir
import concourse.tile as tile

F32 = mybir.dt.float32
FP8 = mybir.dt.float8e4

N_CORES = 8
FULL_BATCH = 16
C = 64  # in channels == out channels
H = 224
W = 224
KH = KW = 3
# Sign-plane slot layout (fp8, per partition): [pad, b0 w=224, 0, b1 w=224,
# pad, pad] -> both batch images form one contiguous 450-wide matmul N strip;
# the shared zero column between them keeps the kw shifts exact.  452 cols
# used, padded to 464 (multiple of 16 for DoubleRow AP steps).
SW = 464   # slot stride
SN = 450   # matmul N (448 real output columns + 2 junk)
B0 = 1     # b0 image at cols 1..224
B1 = 226   # b1 image at cols 226..449


def build_nc(n_batch=2, h=H, w=W, g=16, enable_asserts=False):
    """Build the single-core Bass module (same NEFF runs on all 8 cores)."""
    nc = bacc.Bacc(
        "TRN2",
        target_bir_lowering=False,
        debug=False,
        enable_asserts=enable_asserts,
    )
    assert h % 2 == 0
    NV = h // 2          # input row-pair slots, also output units
    assert NV % 2 == 0, "units are evicted in pairs"

    # Packed tensors: [pair, 128, batch, w].
    xp = nc.dram_tensor("xp", [NV, 128, n_batch, w], F32, kind="ExternalInput")
    wraw = nc.dram_tensor("wraw", [C * C * KH * KW, 1], F32, kind="ExternalInput")
    # wt: host-permuted weight copy, layout [i, (kh kw o)] so every tap block
    # [i, o] is a contiguous [64, 64] slab (pure relayout, no arithmetic).
    wt = nc.dram_tensor("wt", [C, KH * KW * C], F32, kind="ExternalInput")
    yp = nc.dram_tensor("yp", [NV, 128, n_batch, w], F32, kind="ExternalOutput")

    xr = xp.ap().rearrange("j p b w -> p j (b w)")   # [128, NV, n_batch*w]
    yr = yp.ap().rearrange("j p b w -> p j (b w)")

    with tile.TileContext(nc) as tc:
        with (
            tc.tile_pool(name="wpool", bufs=1) as wpool,
            tc.tile_pool(name="icpool", bufs=2) as icpool,
            tc.tile_pool(name="pspool", bufs=3, space="PSUM") as pspool,
            tc.tile_pool(name="ocpool", bufs=2) as ocpool,
        ):
            # Input chunk schedule: a small first chunk so signing (and the
            # first matmuls) start as early as possible, then full chunks.
            g0 = min(4, NV)
            starts = [0]
            while starts[-1] + (g0 if len(starts) == 1 else g) < NV:
                starts.append(starts[-1] + (g0 if len(starts) == 1 else g))
            chunk_of = {}
            for ci, s in enumerate(starts):
                e = starts[ci + 1] if ci + 1 < len(starts) else NV
                for j in range(s, e):
                    chunk_of[j] = (ci, s)

            # Prefetch the first input chunk before anything else so the HWDGE
            # ring starts moving data immediately.
            ic = icpool.tile([128, g, n_batch * w], F32, tag="ic", name="ic")
            nc.sync.dma_start(ic[:, 0:g0, :], xr[:, 0:g0, :])

            # ---- weight prep (one-time, tiny; DMAs via SWDGE) ----
            # scale[o] = mean(|w[o, :, :, :]|), O on partitions, duplicated on
            # both partition halves for the [128]-row eviction.
            w2 = wpool.tile([128, 576], F32)
            wr = wraw.ap().rearrange("(o f) one -> o (f one)", o=C)
            nc.gpsimd.dma_start(w2[0:64], wr)
            nc.gpsimd.dma_start(w2[64:128], wr)
            absw = wpool.tile([128, 576], F32)
            sc_sum = wpool.tile([128, 1], F32)
            nc.scalar.activation(
                out=absw[:], in_=w2[:], func=mybir.ActivationFunctionType.Abs,
                accum_out=sc_sum[:],
            )
            sc128 = wpool.tile([128, 1], F32)
            nc.scalar.mul(sc128[:], sc_sum[:], 1.0 / 576.0)

            # sign(w)^T in fp8, duplicated on both partition halves.
            wtf = wpool.tile([128, 576], F32)
            nc.gpsimd.dma_start(wtf[0:64], wt.ap())
            nc.gpsimd.dma_start(wtf[64:128], wt.ap())
            swt = wpool.tile([128, 576], FP8)
            nc.scalar.sign(swt[:], wtf[:])

            # Resident sign plane [128, NV+1, SW] fp8; slot NV = V_0 copy.
            # Zero the pad columns first (DVE program order: these must not
            # queue behind the weight-block copies, which wait on swt):
            # col 0 (left pad), col 225 (separator / b0 right pad), cols
            # 450-451 (right pads, also read by the junk output column).
            assert n_batch == 2
            plane = wpool.tile([128, NV + 1, SW], FP8)
            nc.vector.memset(plane[:, :, 0:1], 0.0)
            nc.vector.memset(plane[:, :, 225:226], 0.0)
            nc.vector.memset(plane[:, :, 450:452], 0.0)

            def tap(kh, kw):
                o0 = (kh * 3 + kw) * 64
                return slice(o0, o0 + 64)

            # DoubleRow stationary operands [128, 2, 128]: index 1 of the
            # middle dim is the second virtual-K block.
            def cp(dst, src):
                nc.vector.tensor_copy(out=dst, in_=src)

            wdr, wb = [], []
            for kw in range(3):
                t = wpool.tile([128, 2, 128], FP8, name=f"wdr{kw}")
                nc.vector.memset(t[:], 0.0)
                # block 0 (slot m):   [[W0, 0], [W1, W0]]
                cp(t[0:64, 0, 0:64], swt[0:64, tap(0, kw)])
                cp(t[64:128, 0, 0:64], swt[64:128, tap(1, kw)])
                cp(t[64:128, 0, 64:128], swt[64:128, tap(0, kw)])
                # block 1 (slot m+1): [[W2, W1], [0, W2]]
                cp(t[0:64, 1, 0:64], swt[0:64, tap(2, kw)])
                cp(t[0:64, 1, 64:128], swt[0:64, tap(1, kw)])
                cp(t[64:128, 1, 64:128], swt[64:128, tap(2, kw)])
                wdr.append(t)
                tb = wpool.tile([128, 2, 128], FP8, name=f"wb{kw}")
                nc.vector.memset(tb[:], 0.0)
                # block 0 (slot NV-1 = V_{NV-1}): [[0, W0], [0, W1]] (row h-1)
                cp(tb[0:64, 0, 64:128], swt[0:64, tap(0, kw)])
                cp(tb[64:128, 0, 64:128], swt[64:128, tap(1, kw)])
                # block 1 (slot NV = copy of V_0): [[W1, 0], [W2, 0]] (row 0)
                cp(tb[0:64, 1, 0:64], swt[0:64, tap(1, kw)])
                cp(tb[64:128, 1, 0:64], swt[64:128, tap(2, kw)])
                wb.append(tb)

            def rhs(j, kw):
                return plane[:, j : j + 2, kw : kw + SN]

            def evict(ps, oc, jj):
                # psum cols 0..223 = b0, 225..448 = b1 (stride-225 blocks)
                nc.vector.tensor_scalar_mul(
                    oc[:, jj : jj + 2, :].rearrange(
                        "p j (b w) -> p j b w", b=n_batch
                    ),
                    ps[:, :, 0:450].rearrange("p u (b w) -> p u b w", w=225)[
                        :, :, :, 0:w
                    ],
                    sc128[:],
                )

            og = min(8, NV)  # output chunk size (earlier, shorter stores)
            assert og % 2 == 0
            oc = None
            oc_m0 = 0
            ps = None

            for j in range(NV):
                ci, cstart = chunk_of[j]
                if j == cstart and j > 0:
                    gc = min(g, NV - j)
                    ic = icpool.tile([128, g, n_batch * w], F32, tag="ic", name="ic")
                    nc.sync.dma_start(ic[:, 0:gc, :], xr[:, j : j + gc, :])

                nc.scalar.sign(
                    plane[:, j, 1:451].rearrange("p (b w) -> p b w", w=225)[
                        :, :, 0:w
                    ],
                    ic[:, j - cstart, :].rearrange("p (b w) -> p b w", b=n_batch),
                )
                if j == 0:
                    cp(plane[:, NV, 0:452], plane[:, 0, 0:452])

                if j >= 1:
                    m = j - 1  # interior unit -> output rows (2m+1, 2m+2)
                    if m % og == 0:
                        oc = ocpool.tile(
                            [128, og, n_batch * w], F32, tag="oc", name="oc"
                        )
                        oc_m0 = m
                    if m % 2 == 0:
                        # per-unit stride padded to one full PSUM bank (2 KB)
                        ps = pspool.tile([128, 2, 512], F32, tag="ps", name="ps")
                    for kw in range(3):
                        nc.tensor.matmul(
                            ps[:, m % 2, 0:SN], wdr[kw][:], rhs(m, kw),
                            start=(kw == 0), stop=(kw == 2),
                            perf_mode=mybir.MatmulPerfMode.DoubleRow,
                        )
                    if m % 2 == 1:
                        evict(ps, oc, m - 1 - oc_m0)
                    if m == oc_m0 + og - 1:
                        nc.gpsimd.dma_start(yr[:, oc_m0 : m + 1, :], oc[:, 0:og, :])

            # Boundary unit (unit NV-1): rows 0 and h-1 via slots NV-1 and NV.
            m = NV - 1
            if m % og == 0:
                oc = ocpool.tile([128, og, n_batch * w], F32, tag="oc", name="oc")
                oc_m0 = m
            if m % 2 == 0:
                ps = pspool.tile([128, 2, 512], F32, tag="ps", name="ps")
            for kw in range(3):
                nc.tensor.matmul(
                    ps[:, m % 2, 0:SN], wb[kw][:], rhs(NV - 1, kw),
                    start=(kw == 0), stop=(kw == 2),
                    perf_mode=mybir.MatmulPerfMode.DoubleRow,
                )
            if m % 2 == 1:
                evict(ps, oc, m - 1 - oc_m0)
            else:
                # odd NV: evict the single last unit
                nc.vector.tensor_scalar_mul(
                    oc[:, m - oc_m0 : m - oc_m0 + 1, :].rearrange(
                        "p j (b w) -> p j b w", b=n_batch
                    ),
                    ps[:, m % 2 : m % 2 + 1, 0:450].rearrange(
                        "p u (b w) -> p u b w", w=225
                    )[:, :, :, 0:w],
                    sc128[:],
                )
            nc.gpsimd.dma_start(yr[:, oc_m0 : m + 1, :], oc[:, 0 : m - oc_m0 + 1, :])

    nc.compile()
    return nc


_NC_CACHE = {}


def _get_nc(key=(2, H, W, 16)):
    if key not in _NC_CACHE:
        _NC_CACHE[key] = build_nc(*key)
    return _NC_CACHE[key]


def _make_wt(weights):
    # [o*i*kh*kw, 1] -> [i, (kh kw o)] contiguous (pure relayout on host)
    w4 = np.ascontiguousarray(
        weights.reshape(C, C, KH, KW).transpose(1, 2, 3, 0)
    ).reshape(C, KH * KW * C)
    return w4.astype(np.float32, copy=False)


def pack_x(x_shard, h=H, w=W):
    """[nb, C, h, w] -> [h/2, 128, nb, w]; p = parity*64 + channel."""
    nb = x_shard.shape[0]
    xs = x_shard.reshape(nb, C, h // 2, 2, w)
    return np.ascontiguousarray(xs.transpose(2, 3, 1, 0, 4)).reshape(
        h // 2, 128, nb, w
    )


def unpack_y(ypk, h=H, w=W):
    """[h/2, 128, nb, w] -> [nb, C, h, w] per the unit layout."""
    NV = h // 2
    nb = ypk.shape[2]
    y = np.empty((nb, C, h, w), np.float32)
    # interior units m=0..NV-2 -> rows 2m+1 (p<64) and 2m+2 (p>=64)
    interior = ypk[: NV - 1].reshape(NV - 1, 2, C, nb, w)
    y[:, :, 1 : h - 1, :] = interior.transpose(3, 2, 0, 1, 4).reshape(
        nb, C, h - 2, w
    )
    # boundary unit: p<64 -> row 0, p>=64 -> row h-1
    y[:, :, 0, :] = ypk[NV - 1, 0:C].transpose(1, 0, 2)
    y[:, :, h - 1, :] = ypk[NV - 1, C:128].transpose(1, 0, 2)
    return y


def make_in_maps(x, weights):
    x = np.asarray(x, dtype=np.float32)
    weights = np.asarray(weights, dtype=np.float32)
    wt = _make_wt(weights)
    nb = FULL_BATCH // N_CORES
    return [
        {
            "xp": pack_x(x[c * nb : (c + 1) * nb]),
            "wraw": weights,
            "wt": wt,
        }
        for c in range(N_CORES)
    ]


def gather_out(results):
    return np.concatenate([unpack_y(r["yp"]) for r in results], axis=0)


def kernel(x, weights):
    from concourse import bass_utils

    nc = _get_nc()
    in_maps = make_in_maps(x, weights)
    res = bass_utils.run_bass_kernel_spmd(nc, in_maps, core_ids=list(range(N_CORES)))
    return gather_out(res.results)
